# revision 10
# baseline (speedup 1.0000x reference)
"""Causal self-attention on 8 NeuronCores (TRN2), tensor-parallel over heads.

Reference: y = proj(softmax(causal(Q K^T / sqrt(64))) V) with
B=4, T=2048, D=1024, H=16 heads, head_dim=64.

Sharding: each core owns 2 heads (a 128-column slice of the Q/K/V
projections and the matching 128 rows of w_proj) for all batches. Each
core emits a partial [B*T, D] output (bf16); the host sums the 8
partials in fp32 (row-parallel matmul unshard) and reshapes to [B,T,D].

Design notes:
  - all matmul operands bf16 (FWL weight loads, half DMA traffic)
  - weight-stationary Q/K projection (2 PSUM banks, LDW amortized)
  - V projected directly token-major (x tiles stationary, w_v moving)
    so no PE transposes are needed for the AV lhsT
  - K^T stored block-diagonal (kt2): head h's 64 d-rows at partition
    rows h*64 of column block 256j+128h, zeros elsewhere, so S^T runs
    as full-K=128 matmuls that qualify for fast weight load
  - softmax normalization deferred: unnormalized O and the denominator
    row come out of the AV matmuls (ones column trick); 1/den via
    reciprocal_approx_fast, partition-broadcast, divide fused into the
    PSUM->SBUF evacuation multiply
  - jq iterated descending so attention starts with the deepest kk run
    (pipeline fills; the shallow jq=0 block lands where out-proj/qkv
    of neighbor batches provide PE filler work)
  - out-proj evacuation split across DVE and ACT
"""

import sys

for _p in ("/opt/trn_rl_repo",):
    if _p not in sys.path:
        sys.path.insert(0, _p)

import ml_dtypes
import numpy as np

import concourse.bass as bass
import concourse.bacc as bacc
import concourse.mybir as mybir
from concourse import tile
from concourse.bass_utils import run_bass_kernel_spmd

B, T, D, H = 4, 2048, 1024, 16
HD = D // H           # 64 head dim
NCORES = 8
HPC = H // NCORES     # 2 heads per core
CW = HPC * HD         # 128: per-core qkv column slice width
BT = B * T            # 8192 tokens
KC = D // 128         # 8 contraction chunks for the qkv projection
NQ = 512              # query chunk
NG = NQ // 128        # 4 key-tiles per S^T group
F32 = mybir.dt.float32
BF16 = mybir.dt.bfloat16
EXP = mybir.ActivationFunctionType.Exp
BF = ml_dtypes.bfloat16

VST = HPC * (HD + 1)  # 130: V tile stride (per head: 64 cols + ones col)
NKK = T // 128        # 16 key tiles per batch
VBW = NKK * VST + 64  # vb width incl. tail pad for the 128-wide AV ldweights


def build_kernel():
    nc = bacc.Bacc("TRN2", target_bir_lowering=False, debug=False)

    xT = nc.dram_tensor("xT", [D, BT], BF16, kind="ExternalInput")
    # wqkv packed on host as [128, KC, 3*CW]: (kc,:) = rows kc*128..+128 of
    # [w_q_slice | w_k_slice | w_v_slice]
    wqkv = nc.dram_tensor("wqkv", [128, KC * 3 * CW], BF16, kind="ExternalInput")
    wp = nc.dram_tensor("wp", [CW, D], BF16, kind="ExternalInput")
    out = nc.dram_tensor("out", [BT, D], BF16, kind="ExternalOutput")

    with tile.TileContext(nc) as tc:
        _body(tc, xT.ap(), wqkv.ap(), wp.ap(), out.ap())
    nc.compile()
    return nc


def _body(tc, xT, wqkv, wp, out):
    nc = tc.nc
    with (
        tc.tile_pool(name="const", bufs=1) as const,
        tc.tile_pool(name="xin", bufs=2) as xin,
        tc.tile_pool(name="qk", bufs=2) as qkpool,
        tc.tile_pool(name="vb", bufs=2) as vbpool,
        tc.tile_pool(name="pt", bufs=3) as ptpool,
        tc.tile_pool(name="yt", bufs=2) as ytpool,
        tc.tile_pool(name="dn", bufs=2) as dnpool,
        tc.tile_pool(name="os", bufs=3) as ospool,
        tc.tile_pool(name="psA", bufs=2, space="PSUM") as psA,
        tc.tile_pool(name="pst", bufs=2, space="PSUM") as pst,
        tc.tile_pool(name="pav", bufs=1, space="PSUM") as pav,
    ):
        # ---- constants ----
        # per-kc weight loads so the first projection matmul can start as
        # soon as (wqkv chunk 0, x chunk 0) land instead of the full 1MB
        wq_sb = const.tile([128, KC, 3 * CW], BF16, tag="wqkv")
        wq_dr = wqkv.rearrange("p (k c) -> p k c", k=KC)
        for kc in range(KC):
            nc.sync.dma_start(wq_sb[:, kc, :], wq_dr[:, kc, :])
        wp_sb = const.tile([CW, D], BF16, tag="wp")
        nc.sync.dma_start(wp_sb[:], wp[:])
        ones32 = const.tile([128, NKK * HPC], BF16, tag="ones32")
        nc.gpsimd.memset(ones32[:], 1.0)
        scale = 1.0 / float(np.sqrt(HD))

        def qkv_proj(b, qt, kt2, vb):
            tok0 = b * T
            xt = xin.tile([128, KC, T], BF16, tag="xt")
            for kc in range(KC):
                nc.sync.dma_start(
                    xt[:, kc, :],
                    xT[kc * 128 : (kc + 1) * 128, tok0 : tok0 + T],
                )
            # zero kt2's off-diagonal quadrants (cheap on GpSimd, early so
            # it's off the critical path)
            for h in range(HPC):
                ksl = kt2[h * HD : (h + 1) * HD, :]
                z = bass.AP(
                    ksl.tensor,
                    ksl.offset + (1 - h) * 128,
                    [ksl.ap[0], [256, NKK], [1, 128]],
                )
                nc.gpsimd.memset(z, 0.0)
            # Q^T and K^T, weight-stationary over kc, 2 PSUM banks
            for m in range(2):
                for g in range(2):
                    ps0 = psA.tile([128, NQ], F32, tag="ps")
                    ps1 = psA.tile([128, NQ], F32, tag="ps")
                    t0 = g * 2 * NQ
                    for kc in range(KC):
                        w_ap = wq_sb[:, kc, m * CW : (m + 1) * CW]
                        nc.tensor.matmul(
                            ps0[:], w_ap, xt[:, kc, t0 : t0 + NQ],
                            start=(kc == 0), stop=(kc == KC - 1),
                        )
                        nc.tensor.matmul(
                            ps1[:], w_ap, xt[:, kc, t0 + NQ : t0 + 2 * NQ],
                            start=(kc == 0), stop=(kc == KC - 1),
                        )
                    if m == 0:
                        nc.vector.tensor_copy(qt[:, t0 : t0 + NQ], ps0[:])
                        nc.vector.tensor_copy(qt[:, t0 + NQ : t0 + 2 * NQ], ps1[:])
                    else:
                        # K^T into block-diagonal kt2
                        for ci, ps in ((0, ps0), (1, ps1)):
                            j0 = (t0 + ci * NQ) // 128
                            for h in range(HPC):
                                ksl = kt2[h * HD : (h + 1) * HD, :]
                                dstk = bass.AP(
                                    ksl.tensor,
                                    ksl.offset + j0 * 256 + h * 128,
                                    [ksl.ap[0], [256, NG], [1, 128]],
                                )
                                psl = ps[h * HD : (h + 1) * HD, :]
                                srck = bass.AP(
                                    psl.tensor,
                                    psl.offset,
                                    [psl.ap[0], [128, NG], [1, 128]],
                                )
                                nc.vector.tensor_copy(dstk, srck)
            # V^T token-major: x tiles stationary, w_v moving; out [tok, c]
            for g in range(NKK // 4):
                psv = psA.tile([128, NQ], F32, tag="ps")
                for tt in range(4):
                    kk = g * 4 + tt
                    for kc in range(KC):
                        nc.tensor.matmul(
                            psv[:, tt * 128 : (tt + 1) * 128],
                            xt[:, kc, kk * 128 : (kk + 1) * 128],
                            wq_sb[:, kc, 2 * CW : 3 * CW],
                            start=(kc == 0), stop=(kc == KC - 1),
                        )
                # scatter 4 token-tiles into vb's 65-stride head blocks
                dstv = bass.AP(
                    vb.tensor,
                    vb[:].offset + g * 4 * VST,
                    [vb[:].ap[0], [VST, 4], [HD + 1, HPC], [1, HD]],
                )
                srcv = psv[:].rearrange("p (t h d) -> p t h d", t=4, h=HPC)
                nc.vector.tensor_copy(dstv, srcv)
            # ones columns (denominator trick): col 65*j + HD of each block
            onesv = bass.AP(
                vb.tensor,
                vb[:].offset + HD,
                [vb[:].ap[0], [HD + 1, NKK * HPC]],
            )
            nc.vector.tensor_copy(onesv, ones32[:])
            # tail pad so the 128-wide AV ldweights never reads junk
            padv = bass.AP(
                vb.tensor, vb[:].offset + NKK * VST, [vb[:].ap[0], [1, 64]]
            )
            nc.gpsimd.memset(padv, 0.0)

        def attention(b, qt, kt2, vb, yt):
            # flattened (jq, kk) stream, jq descending, software-pipelined
            # one S^T step ahead so the PE queue never head-of-line blocks
            steps = []
            for jq in range(T // NQ - 1, -1, -1):
                for kk in range(NG * (jq + 1)):
                    steps.append((jq, kk))

            def s_step(jq, kk):
                q0 = jq * NQ
                i = kk - NG * jq        # >= 0 on the diagonal run
                c0 = max(i, 0) * 128    # first valid q col in this chunk
                w = NQ - c0
                st = pst.tile([128, HPC * NQ], F32, tag="st")
                for h in range(HPC):
                    nc.tensor.matmul(
                        st[:, h * NQ + c0 : (h + 1) * NQ],
                        kt2[:, kk * 256 + h * 128 : kk * 256 + (h + 1) * 128],
                        qt[:, q0 + c0 : q0 + NQ],
                        start=True,
                        stop=True,
                    )
                return st, c0, w, i

            def e_step(st, c0, w, i):
                ptk = ptpool.tile([128, HPC * NQ], BF16, tag="pt")
                stv = bass.AP(st.tensor, st[:].offset + c0,
                              [st[:].ap[0], [NQ, HPC], [1, w]])
                ptv = bass.AP(ptk.tensor, ptk[:].offset + c0,
                              [ptk[:].ap[0], [NQ, HPC], [1, w]])
                nc.scalar.activation(ptv, stv, EXP, scale=scale)
                if i >= 0:
                    # zero q < kpart inside the 128-wide diagonal block
                    tri = bass.AP(ptk.tensor, ptk[:].offset + c0,
                                  [ptk[:].ap[0], [NQ, HPC], [1, 128]])
                    nc.gpsimd.affine_select(
                        out=tri,
                        in_=tri,
                        pattern=[[0, HPC], [1, 128]],
                        channel_multiplier=-1,
                        base=0,
                        compare_op=mybir.AluOpType.is_ge,
                        fill=0.0,
                    )
                return ptk

            def av_step(avs, ptk, c0, kk, nkk):
                for h in range(HPC):
                    # 128-wide stationary: head h's 65 cols + 63 junk
                    nc.tensor.matmul(
                        avs[h][:, c0:NQ],
                        vb[:, kk * VST + h * (HD + 1) :
                             kk * VST + h * (HD + 1) + 128],
                        ptk[:, h * NQ + c0 : (h + 1) * NQ],
                        start=(kk == 0),
                        stop=(kk == nkk - 1),
                    )

            def norm_evac(avs, jq):
                # rows 0..63 unnormalized O^T, row 64 denominator.
                # y = O * (1/den) fused into the evacuation multiply.
                q0 = jq * NQ
                for h in range(HPC):
                    # custom DVE ops don't honor a nonzero base partition on
                    # the input AP: stage the den row to partition 0 first
                    dnr = dnpool.tile([1, NQ], F32, tag=f"d{h}")
                    nc.vector.tensor_copy(dnr[:], avs[h][HD : HD + 1, :])
                    r = dnpool.tile([1, NQ], F32, tag=f"r{h}")
                    nc.vector.reciprocal_approx_fast(r[:], dnr[:])
                    R = dnpool.tile([HD, NQ], F32, tag=f"R{h}")
                    nc.gpsimd.partition_broadcast(R[:], r[:])
                    nc.vector.tensor_mul(
                        yt[h * HD : (h + 1) * HD, q0 : q0 + NQ],
                        avs[h][0:HD, :],
                        R[:],
                    )

            def out_proj_chunk(jq):
                # out-proj for this jq's 512 tokens, emitted right after its
                # normalize: fills PE stalls inside attention and spreads the
                # evacuation + output DMA instead of a serial tail phase
                tok0 = b * T + jq * NQ
                for ts in range(NQ // 128):
                    col = jq * NQ + ts * 128
                    os_ = ospool.tile([128, D], BF16, tag="os")
                    for nn in range(D // NQ):
                        pp = psA.tile([128, NQ], F32, tag="ps")
                        nc.tensor.matmul(
                            pp[:],
                            yt[:, col : col + 128],
                            wp_sb[:, nn * NQ : (nn + 1) * NQ],
                            start=True,
                            stop=True,
                        )
                        # split evacuation across DVE and ACT
                        if nn == 0:
                            nc.vector.tensor_copy(
                                os_[:, nn * NQ : (nn + 1) * NQ], pp[:]
                            )
                        else:
                            nc.scalar.copy(os_[:, nn * NQ : (nn + 1) * NQ], pp[:])
                    nc.sync.dma_start(
                        out[tok0 + ts * 128 : tok0 + (ts + 1) * 128, :], os_[:]
                    )

            avs = None
            pend = s_step(*steps[0])
            for n, (jq, kk) in enumerate(steps):
                if kk == 0:
                    av0 = pav.tile([128, NQ], F32, tag="av0")
                    av1 = pav.tile([128, NQ], F32, tag="av1")
                    avs = (av0, av1)
                ptk = e_step(*pend)
                c0 = pend[1]
                if n + 1 < len(steps):
                    pend = s_step(*steps[n + 1])
                nkk = NG * (jq + 1)
                av_step(avs, ptk, c0, kk, nkk)
                if kk == nkk - 1:
                    norm_evac(avs, jq)
                    out_proj_chunk(jq)

        for b in range(B):
            qt = qkpool.tile([128, T], BF16, tag="qt")
            kt2 = qkpool.tile([128, 2 * T], BF16, tag="kt2")
            vb = vbpool.tile([128, VBW], BF16, tag="vb")
            yt = ytpool.tile([128, T], BF16, tag="yt")
            qkv_proj(b, qt, kt2, vb)
            attention(b, qt, kt2, vb, yt)


_NC_CACHE = None


def make_in_maps(x, w_attn, w_proj):
    x = np.asarray(x, dtype=np.float32)
    w_attn = np.asarray(w_attn, dtype=np.float32)
    w_proj = np.asarray(w_proj, dtype=np.float32)

    xT = np.ascontiguousarray(x.reshape(BT, D).T).astype(BF)  # [D, BT]

    in_maps = []
    for c in range(NCORES):
        c0 = c * CW
        wq = w_attn[:, c0 : c0 + CW]
        wk = w_attn[:, D + c0 : D + c0 + CW]
        wv = w_attn[:, 2 * D + c0 : 2 * D + c0 + CW]
        wslice = np.concatenate([wq, wk, wv], axis=1)          # [D, 3*CW]
        wpacked = np.ascontiguousarray(
            wslice.reshape(KC, 128, 3 * CW).transpose(1, 0, 2)
        ).reshape(128, KC * 3 * CW).astype(BF)
        wpc = np.ascontiguousarray(w_proj[c0 : c0 + CW, :]).astype(BF)
        in_maps.append({"xT": xT, "wqkv": wpacked, "wp": wpc})
    return in_maps


def kernel(x: np.ndarray, w_attn: np.ndarray, w_proj: np.ndarray) -> np.ndarray:
    global _NC_CACHE
    if _NC_CACHE is None:
        _NC_CACHE = build_kernel()
    nc = _NC_CACHE

    in_maps = make_in_maps(x, w_attn, w_proj)
    res = run_bass_kernel_spmd(nc, in_maps, core_ids=list(range(NCORES)))
    acc = np.zeros((BT, D), dtype=np.float32)
    for r in res.results:
        acc += np.asarray(r["out"], dtype=np.float32)
    return acc.reshape(B, T, D)


if __name__ == "__main__":
    inputs = {
        "x": np.random.randn(B, T, D).astype(np.float32),
        "w_attn": (np.random.randn(D, 3 * D) / np.sqrt(D)).astype(np.float32),
        "w_proj": (np.random.randn(D, D) / np.sqrt(D)).astype(np.float32),
    }
    y = kernel(**inputs)
    print(y.shape, y.dtype)


# revision 12
# speedup vs baseline: 1.0517x; 1.0517x over previous
"""Causal self-attention on 8 NeuronCores (TRN2), tensor-parallel over heads.

Reference: y = proj(softmax(causal(Q K^T / sqrt(64))) V) with
B=4, T=2048, D=1024, H=16 heads, head_dim=64.

Sharding: each core owns 2 heads (a 128-column slice of the Q/K/V
projections and the matching 128 rows of w_proj) for all batches. Each
core emits a partial [B*T, D] output (bf16); the host sums the 8
partials in fp32 (row-parallel matmul unshard) and reshapes to [B,T,D].

Design notes:
  - all matmul operands bf16 (FWL weight loads, half DMA traffic)
  - weight-stationary Q/K projection (2 PSUM banks, LDW amortized)
  - V projected directly token-major (x tiles stationary, w_v moving)
    so no PE transposes are needed for the AV lhsT
  - K^T stored block-diagonal (kt2): head h's 64 d-rows at partition
    rows h*64 of column block 256j+128h, zeros elsewhere, so S^T runs
    as full-K=128 matmuls that qualify for fast weight load
  - softmax normalization deferred: unnormalized O and the denominator
    row come out of the AV matmuls (ones column trick); 1/den via
    reciprocal_approx_fast, partition-broadcast, divide fused into the
    PSUM->SBUF evacuation multiply
  - jq iterated descending so attention starts with the deepest kk run
    (pipeline fills; the shallow jq=0 block lands where out-proj/qkv
    of neighbor batches provide PE filler work)
  - out-proj evacuation split across DVE and ACT
"""

import sys

for _p in ("/opt/trn_rl_repo",):
    if _p not in sys.path:
        sys.path.insert(0, _p)

import ml_dtypes
import numpy as np

import concourse.bass as bass
import concourse.bacc as bacc
import concourse.mybir as mybir
from concourse import tile
from concourse.bass_utils import run_bass_kernel_spmd

B, T, D, H = 4, 2048, 1024, 16
HD = D // H           # 64 head dim
NCORES = 8
HPC = H // NCORES     # 2 heads per core
CW = HPC * HD         # 128: per-core qkv column slice width
BT = B * T            # 8192 tokens
KC = D // 128         # 8 contraction chunks for the qkv projection
NQ = 512              # query chunk
NG = NQ // 128        # 4 key-tiles per S^T group
F32 = mybir.dt.float32
BF16 = mybir.dt.bfloat16
EXP = mybir.ActivationFunctionType.Exp
BF = ml_dtypes.bfloat16

VST = HPC * (HD + 1)  # 130: V tile stride (per head: 64 cols + ones col)
NKK = T // 128        # 16 key tiles per batch
VBW = NKK * VST + 64  # vb width incl. tail pad for the 128-wide AV ldweights


def build_kernel():
    nc = bacc.Bacc("TRN2", target_bir_lowering=False, debug=False)

    xT = nc.dram_tensor("xT", [D, BT], BF16, kind="ExternalInput")
    # wqkv packed on host as [128, KC, 3*CW]: (kc,:) = rows kc*128..+128 of
    # [w_q_slice | w_k_slice | w_v_slice]
    wqkv = nc.dram_tensor("wqkv", [128, KC * 3 * CW], BF16, kind="ExternalInput")
    wp = nc.dram_tensor("wp", [CW, D], BF16, kind="ExternalInput")
    out = nc.dram_tensor("out", [BT, D], BF16, kind="ExternalOutput")

    with tile.TileContext(nc) as tc:
        _body(tc, xT.ap(), wqkv.ap(), wp.ap(), out.ap())
    nc.compile()
    return nc


def _body(tc, xT, wqkv, wp, out):
    nc = tc.nc
    with (
        tc.tile_pool(name="const", bufs=1) as const,
        tc.tile_pool(name="xin", bufs=2) as xin,
        tc.tile_pool(name="qk", bufs=2) as qkpool,
        tc.tile_pool(name="vb", bufs=2) as vbpool,
        tc.tile_pool(name="pt", bufs=3) as ptpool,
        tc.tile_pool(name="yt", bufs=2) as ytpool,
        tc.tile_pool(name="dn", bufs=2) as dnpool,
        tc.tile_pool(name="os", bufs=3) as ospool,
        tc.tile_pool(name="psA", bufs=2, space="PSUM") as psA,
        tc.tile_pool(name="pst", bufs=2, space="PSUM") as pst,
        tc.tile_pool(name="pav", bufs=1, space="PSUM") as pav,
    ):
        # ---- constants ----
        # per-kc weight loads so the first projection matmul can start as
        # soon as (wqkv chunk 0, x chunk 0) land instead of the full 1MB
        wq_sb = const.tile([128, KC, 3 * CW], BF16, tag="wqkv")
        wq_dr = wqkv.rearrange("p (k c) -> p k c", k=KC)
        for kc in range(KC):
            nc.sync.dma_start(wq_sb[:, kc, :], wq_dr[:, kc, :])
        wp_sb = const.tile([CW, D], BF16, tag="wp")
        nc.sync.dma_start(wp_sb[:], wp[:])
        ones32 = const.tile([128, NKK * HPC], BF16, tag="ones32")
        nc.gpsimd.memset(ones32[:], 1.0)
        scale = 1.0 / float(np.sqrt(HD))

        def qkv_proj(b, qt, kt2, vb):
            tok0 = b * T
            xt = xin.tile([128, KC, T], BF16, tag="xt")
            # 128KB segments, seg-major: one segment per DMA queue in
            # parallel, so the first projection matmul starts ~5us in
            for s in range(4):
                for kc in range(KC):
                    nc.sync.dma_start(
                        xt[:, kc, s * NQ : (s + 1) * NQ],
                        xT[kc * 128 : (kc + 1) * 128,
                           tok0 + s * NQ : tok0 + (s + 1) * NQ],
                    )
            # zero kt2's off-diagonal quadrants (cheap on GpSimd, early so
            # it's off the critical path)
            for h in range(HPC):
                ksl = kt2[h * HD : (h + 1) * HD, :]
                z = bass.AP(
                    ksl.tensor,
                    ksl.offset + (1 - h) * 128,
                    [ksl.ap[0], [256, NKK], [1, 128]],
                )
                nc.gpsimd.memset(z, 0.0)
            # Q^T and K^T, weight-stationary over kc, 2 PSUM banks
            for m in range(2):
                for g in range(2):
                    ps0 = psA.tile([128, NQ], F32, tag="ps")
                    ps1 = psA.tile([128, NQ], F32, tag="ps")
                    t0 = g * 2 * NQ
                    for kc in range(KC):
                        w_ap = wq_sb[:, kc, m * CW : (m + 1) * CW]
                        nc.tensor.matmul(
                            ps0[:], w_ap, xt[:, kc, t0 : t0 + NQ],
                            start=(kc == 0), stop=(kc == KC - 1),
                        )
                        nc.tensor.matmul(
                            ps1[:], w_ap, xt[:, kc, t0 + NQ : t0 + 2 * NQ],
                            start=(kc == 0), stop=(kc == KC - 1),
                        )
                    if m == 0:
                        nc.vector.tensor_copy(qt[:, t0 : t0 + NQ], ps0[:])
                        nc.vector.tensor_copy(qt[:, t0 + NQ : t0 + 2 * NQ], ps1[:])
                    else:
                        # K^T into block-diagonal kt2
                        for ci, ps in ((0, ps0), (1, ps1)):
                            j0 = (t0 + ci * NQ) // 128
                            for h in range(HPC):
                                ksl = kt2[h * HD : (h + 1) * HD, :]
                                dstk = bass.AP(
                                    ksl.tensor,
                                    ksl.offset + j0 * 256 + h * 128,
                                    [ksl.ap[0], [256, NG], [1, 128]],
                                )
                                psl = ps[h * HD : (h + 1) * HD, :]
                                srck = bass.AP(
                                    psl.tensor,
                                    psl.offset,
                                    [psl.ap[0], [128, NG], [1, 128]],
                                )
                                nc.vector.tensor_copy(dstk, srck)
            # V^T token-major: x tiles stationary, w_v moving; out [tok, c]
            for g in range(NKK // 4):
                psv = psA.tile([128, NQ], F32, tag="ps")
                for tt in range(4):
                    kk = g * 4 + tt
                    for kc in range(KC):
                        nc.tensor.matmul(
                            psv[:, tt * 128 : (tt + 1) * 128],
                            xt[:, kc, kk * 128 : (kk + 1) * 128],
                            wq_sb[:, kc, 2 * CW : 3 * CW],
                            start=(kc == 0), stop=(kc == KC - 1),
                        )
                # scatter 4 token-tiles into vb's 65-stride head blocks
                dstv = bass.AP(
                    vb.tensor,
                    vb[:].offset + g * 4 * VST,
                    [vb[:].ap[0], [VST, 4], [HD + 1, HPC], [1, HD]],
                )
                srcv = psv[:].rearrange("p (t h d) -> p t h d", t=4, h=HPC)
                nc.vector.tensor_copy(dstv, srcv)
            # ones columns (denominator trick): col 65*j + HD of each block
            onesv = bass.AP(
                vb.tensor,
                vb[:].offset + HD,
                [vb[:].ap[0], [HD + 1, NKK * HPC]],
            )
            nc.vector.tensor_copy(onesv, ones32[:])
            # tail pad so the 128-wide AV ldweights never reads junk
            padv = bass.AP(
                vb.tensor, vb[:].offset + NKK * VST, [vb[:].ap[0], [1, 64]]
            )
            nc.gpsimd.memset(padv, 0.0)

        def attention(b, qt, kt2, vb, yt):
            # flattened (jq, kk) stream, jq descending, software-pipelined
            # one S^T step ahead so the PE queue never head-of-line blocks
            steps = []
            for jq in range(T // NQ - 1, -1, -1):
                for kk in range(NG * (jq + 1)):
                    steps.append((jq, kk))

            def s_step(jq, kk):
                q0 = jq * NQ
                i = kk - NG * jq        # >= 0 on the diagonal run
                c0 = max(i, 0) * 128    # first valid q col in this chunk
                w = NQ - c0
                st = pst.tile([128, HPC * NQ], F32, tag="st")
                for h in range(HPC):
                    nc.tensor.matmul(
                        st[:, h * NQ + c0 : (h + 1) * NQ],
                        kt2[:, kk * 256 + h * 128 : kk * 256 + (h + 1) * 128],
                        qt[:, q0 + c0 : q0 + NQ],
                        start=True,
                        stop=True,
                    )
                return st, c0, w, i

            def e_step(st, c0, w, i):
                ptk = ptpool.tile([128, HPC * NQ], BF16, tag="pt")
                stv = bass.AP(st.tensor, st[:].offset + c0,
                              [st[:].ap[0], [NQ, HPC], [1, w]])
                ptv = bass.AP(ptk.tensor, ptk[:].offset + c0,
                              [ptk[:].ap[0], [NQ, HPC], [1, w]])
                nc.scalar.activation(ptv, stv, EXP, scale=scale)
                if i >= 0:
                    # zero q < kpart inside the 128-wide diagonal block
                    tri = bass.AP(ptk.tensor, ptk[:].offset + c0,
                                  [ptk[:].ap[0], [NQ, HPC], [1, 128]])
                    nc.gpsimd.affine_select(
                        out=tri,
                        in_=tri,
                        pattern=[[0, HPC], [1, 128]],
                        channel_multiplier=-1,
                        base=0,
                        compare_op=mybir.AluOpType.is_ge,
                        fill=0.0,
                    )
                return ptk

            def av_step(avs, ptk, c0, kk, nkk):
                for h in range(HPC):
                    # 128-wide stationary: head h's 65 cols + 63 junk
                    nc.tensor.matmul(
                        avs[h][:, c0:NQ],
                        vb[:, kk * VST + h * (HD + 1) :
                             kk * VST + h * (HD + 1) + 128],
                        ptk[:, h * NQ + c0 : (h + 1) * NQ],
                        start=(kk == 0),
                        stop=(kk == nkk - 1),
                    )

            def norm_evac(avs, jq):
                # rows 0..63 unnormalized O^T, row 64 denominator.
                # y = O * (1/den) fused into the evacuation multiply.
                q0 = jq * NQ
                for h in range(HPC):
                    # custom DVE ops don't honor a nonzero base partition on
                    # the input AP: stage the den row to partition 0 first
                    dnr = dnpool.tile([1, NQ], F32, tag=f"d{h}")
                    nc.vector.tensor_copy(dnr[:], avs[h][HD : HD + 1, :])
                    r = dnpool.tile([1, NQ], F32, tag=f"r{h}")
                    nc.vector.reciprocal_approx_fast(r[:], dnr[:])
                    R = dnpool.tile([HD, NQ], F32, tag=f"R{h}")
                    nc.gpsimd.partition_broadcast(R[:], r[:])
                    nc.vector.tensor_mul(
                        yt[h * HD : (h + 1) * HD, q0 : q0 + NQ],
                        avs[h][0:HD, :],
                        R[:],
                    )

            def out_proj_chunk(jq):
                # out-proj for this jq's 512 tokens, emitted right after its
                # normalize: fills PE stalls inside attention and spreads the
                # evacuation + output DMA instead of a serial tail phase
                tok0 = b * T + jq * NQ
                for ts in range(NQ // 128):
                    col = jq * NQ + ts * 128
                    os_ = ospool.tile([128, D], BF16, tag="os")
                    for nn in range(D // NQ):
                        pp = psA.tile([128, NQ], F32, tag="ps")
                        nc.tensor.matmul(
                            pp[:],
                            yt[:, col : col + 128],
                            wp_sb[:, nn * NQ : (nn + 1) * NQ],
                            start=True,
                            stop=True,
                        )
                        # split evacuation across DVE and ACT
                        if nn == 0:
                            nc.vector.tensor_copy(
                                os_[:, nn * NQ : (nn + 1) * NQ], pp[:]
                            )
                        else:
                            nc.scalar.copy(os_[:, nn * NQ : (nn + 1) * NQ], pp[:])
                    nc.sync.dma_start(
                        out[tok0 + ts * 128 : tok0 + (ts + 1) * 128, :], os_[:]
                    )

            avs = None
            opq = None   # out-proj chunk delayed one jq so its lhsT (yt) is
            pend = s_step(*steps[0])   # ready and the PE queue never blocks
            for n, (jq, kk) in enumerate(steps):
                if kk == 0:
                    av0 = pav.tile([128, NQ], F32, tag="av0")
                    av1 = pav.tile([128, NQ], F32, tag="av1")
                    avs = (av0, av1)
                ptk = e_step(*pend)
                c0 = pend[1]
                if n + 1 < len(steps):
                    pend = s_step(*steps[n + 1])
                nkk = NG * (jq + 1)
                av_step(avs, ptk, c0, kk, nkk)
                if kk == nkk - 1:
                    norm_evac(avs, jq)
                    if opq is not None:
                        out_proj_chunk(opq)
                    opq = jq
            out_proj_chunk(opq)

        for b in range(B):
            qt = qkpool.tile([128, T], BF16, tag="qt")
            kt2 = qkpool.tile([128, 2 * T], BF16, tag="kt2")
            vb = vbpool.tile([128, VBW], BF16, tag="vb")
            yt = ytpool.tile([128, T], BF16, tag="yt")
            qkv_proj(b, qt, kt2, vb)
            attention(b, qt, kt2, vb, yt)


_NC_CACHE = None


def make_in_maps(x, w_attn, w_proj):
    x = np.asarray(x, dtype=np.float32)
    w_attn = np.asarray(w_attn, dtype=np.float32)
    w_proj = np.asarray(w_proj, dtype=np.float32)

    xT = np.ascontiguousarray(x.reshape(BT, D).T).astype(BF)  # [D, BT]

    in_maps = []
    for c in range(NCORES):
        c0 = c * CW
        wq = w_attn[:, c0 : c0 + CW]
        wk = w_attn[:, D + c0 : D + c0 + CW]
        wv = w_attn[:, 2 * D + c0 : 2 * D + c0 + CW]
        wslice = np.concatenate([wq, wk, wv], axis=1)          # [D, 3*CW]
        wpacked = np.ascontiguousarray(
            wslice.reshape(KC, 128, 3 * CW).transpose(1, 0, 2)
        ).reshape(128, KC * 3 * CW).astype(BF)
        wpc = np.ascontiguousarray(w_proj[c0 : c0 + CW, :]).astype(BF)
        in_maps.append({"xT": xT, "wqkv": wpacked, "wp": wpc})
    return in_maps


def kernel(x: np.ndarray, w_attn: np.ndarray, w_proj: np.ndarray) -> np.ndarray:
    global _NC_CACHE
    if _NC_CACHE is None:
        _NC_CACHE = build_kernel()
    nc = _NC_CACHE

    in_maps = make_in_maps(x, w_attn, w_proj)
    res = run_bass_kernel_spmd(nc, in_maps, core_ids=list(range(NCORES)))
    acc = np.zeros((BT, D), dtype=np.float32)
    for r in res.results:
        acc += np.asarray(r["out"], dtype=np.float32)
    return acc.reshape(B, T, D)


if __name__ == "__main__":
    inputs = {
        "x": np.random.randn(B, T, D).astype(np.float32),
        "w_attn": (np.random.randn(D, 3 * D) / np.sqrt(D)).astype(np.float32),
        "w_proj": (np.random.randn(D, D) / np.sqrt(D)).astype(np.float32),
    }
    y = kernel(**inputs)
    print(y.shape, y.dtype)


# revision 17
# speedup vs baseline: 1.1820x; 1.1238x over previous
"""Causal self-attention on 8 NeuronCores (TRN2), tensor-parallel over heads.

Reference: y = proj(softmax(causal(Q K^T / sqrt(64))) V) with
B=4, T=2048, D=1024, H=16 heads, head_dim=64.

Sharding: each core owns 2 heads (a 128-column slice of the Q/K/V
projections and the matching 128 rows of w_proj) for all batches. Each
core emits a partial [B*T, D] output (bf16); the host sums the 8
partials in fp32 (row-parallel matmul unshard) and reshapes to [B,T,D].

Design notes:
  - all matmul operands bf16 (FWL weight loads, half DMA traffic)
  - weight-stationary Q/K projection (2 PSUM banks, LDW amortized)
  - V projected directly token-major (x tiles stationary, w_v moving)
    so no PE transposes are needed for the AV lhsT
  - K^T stored block-diagonal (kt2): head h's 64 d-rows at partition
    rows h*64 of column block 256j+128h, zeros elsewhere, so S^T runs
    as full-K=128 matmuls that qualify for fast weight load
  - softmax normalization deferred: unnormalized O and the denominator
    row come out of the AV matmuls (ones column trick); 1/den via
    reciprocal_approx_fast, partition-broadcast, divide fused into the
    PSUM->SBUF evacuation multiply
  - jq iterated descending so attention starts with the deepest kk run
    (pipeline fills; the shallow jq=0 block lands where out-proj/qkv
    of neighbor batches provide PE filler work)
  - out-proj evacuation split across DVE and ACT
"""

import sys

for _p in ("/opt/trn_rl_repo",):
    if _p not in sys.path:
        sys.path.insert(0, _p)

import ml_dtypes
import numpy as np

import concourse.bass as bass
import concourse.bacc as bacc
import concourse.mybir as mybir
from concourse import tile
from concourse.bass_utils import run_bass_kernel_spmd

B, T, D, H = 4, 2048, 1024, 16
HD = D // H           # 64 head dim
NCORES = 8
HPC = H // NCORES     # 2 heads per core
CW = HPC * HD         # 128: per-core qkv column slice width
BT = B * T            # 8192 tokens
KC = D // 128         # 8 contraction chunks for the qkv projection
NQ = 512              # query chunk
NG = NQ // 128        # 4 key-tiles per S^T group
F32 = mybir.dt.float32
BF16 = mybir.dt.bfloat16
EXP = mybir.ActivationFunctionType.Exp
BF = ml_dtypes.bfloat16

VST = HPC * (HD + 1)  # 130: V tile stride (per head: 64 cols + ones col)
NKK = T // 128        # 16 key tiles per batch
VBW = NKK * VST + 64  # vb width incl. tail pad for the 128-wide AV ldweights


def build_kernel():
    nc = bacc.Bacc("TRN2", target_bir_lowering=False, debug=False)

    xT = nc.dram_tensor("xT", [D, BT], BF16, kind="ExternalInput")
    # wqkv packed on host as [128, KC, 3*CW]: (kc,:) = rows kc*128..+128 of
    # [w_q_slice | w_k_slice | w_v_slice]
    wqkv = nc.dram_tensor("wqkv", [128, KC * 3 * CW], BF16, kind="ExternalInput")
    wp = nc.dram_tensor("wp", [CW, D], BF16, kind="ExternalInput")
    out = nc.dram_tensor("out", [BT, D], BF16, kind="ExternalOutput")

    with tile.TileContext(nc) as tc:
        _body(tc, xT.ap(), wqkv.ap(), wp.ap(), out.ap())
    nc.compile()
    return nc


def _body(tc, xT, wqkv, wp, out):
    nc = tc.nc
    with (
        tc.tile_pool(name="const", bufs=1) as const,
        tc.tile_pool(name="xin", bufs=2) as xin,
        tc.tile_pool(name="qk", bufs=2) as qkpool,
        tc.tile_pool(name="vb", bufs=2) as vbpool,
        tc.tile_pool(name="pt", bufs=3) as ptpool,
        tc.tile_pool(name="yt", bufs=2) as ytpool,
        tc.tile_pool(name="dn", bufs=2) as dnpool,
        tc.tile_pool(name="os", bufs=3) as ospool,
        tc.tile_pool(name="psA", bufs=2, space="PSUM") as psA,
        tc.tile_pool(name="pst", bufs=2, space="PSUM") as pst,
        tc.tile_pool(name="pav", bufs=1, space="PSUM") as pav,
    ):
        # ---- constants ----
        # per-kc weight loads so the first projection matmul can start as
        # soon as (wqkv chunk 0, x chunk 0) land instead of the full 1MB
        wq_sb = const.tile([128, KC, 3 * CW], BF16, tag="wqkv")
        wq_dr = wqkv.rearrange("p (k c) -> p k c", k=KC)
        for kc in range(KC):
            nc.sync.dma_start(wq_sb[:, kc, :], wq_dr[:, kc, :])
        wp_sb = const.tile([CW, D], BF16, tag="wp")
        nc.sync.dma_start(wp_sb[:], wp[:])
        ones32 = const.tile([128, NKK * HPC], BF16, tag="ones32")
        nc.gpsimd.memset(ones32[:], 1.0)
        scale = 1.0 / float(np.sqrt(HD))

        def qkv_emitters(b, qt, kt2, vb):
            """qkv_proj(b) as a list of closures, each emitting one dense
            PE burst (or the DMA prologue), so they can be interleaved into
            the previous batch's attention stream as PE filler work."""
            tok0 = b * T
            xth = []

            def prologue():
                xt = xin.tile([128, KC, T], BF16, tag="xt")
                xth.append(xt)
                # 128KB segments, seg-major: lands one per DMA queue in
                # parallel so the first projection matmul starts early
                for s in range(4):
                    for kc in range(KC):
                        nc.sync.dma_start(
                            xt[:, kc, s * NQ : (s + 1) * NQ],
                            xT[kc * 128 : (kc + 1) * 128,
                               tok0 + s * NQ : tok0 + (s + 1) * NQ],
                        )
                # zero kt2's off-diagonal quadrants (cheap on GpSimd)
                for h in range(HPC):
                    ksl = kt2[h * HD : (h + 1) * HD, :]
                    z = bass.AP(
                        ksl.tensor,
                        ksl.offset + (1 - h) * 128,
                        [ksl.ap[0], [256, NKK], [1, 128]],
                    )
                    nc.gpsimd.memset(z, 0.0)

            def qk_group(m, g):
                def em():
                    xt = xth[0]
                    ps0 = psA.tile([128, NQ], F32, tag="ps")
                    ps1 = psA.tile([128, NQ], F32, tag="ps")
                    t0 = g * 2 * NQ
                    for kc in range(KC):
                        w_ap = wq_sb[:, kc, m * CW : (m + 1) * CW]
                        nc.tensor.matmul(
                            ps0[:], w_ap, xt[:, kc, t0 : t0 + NQ],
                            start=(kc == 0), stop=(kc == KC - 1),
                        )
                        nc.tensor.matmul(
                            ps1[:], w_ap, xt[:, kc, t0 + NQ : t0 + 2 * NQ],
                            start=(kc == 0), stop=(kc == KC - 1),
                        )
                    if m == 0:
                        nc.vector.tensor_copy(qt[:, t0 : t0 + NQ], ps0[:])
                        nc.vector.tensor_copy(qt[:, t0 + NQ : t0 + 2 * NQ], ps1[:])
                    else:
                        # K^T into block-diagonal kt2
                        for ci, ps in ((0, ps0), (1, ps1)):
                            j0 = (t0 + ci * NQ) // 128
                            for h in range(HPC):
                                ksl = kt2[h * HD : (h + 1) * HD, :]
                                dstk = bass.AP(
                                    ksl.tensor,
                                    ksl.offset + j0 * 256 + h * 128,
                                    [ksl.ap[0], [256, NG], [1, 128]],
                                )
                                psl = ps[h * HD : (h + 1) * HD, :]
                                srck = bass.AP(
                                    psl.tensor,
                                    psl.offset,
                                    [psl.ap[0], [128, NG], [1, 128]],
                                )
                                nc.vector.tensor_copy(dstk, srck)
                return em

            def v_group(g):
                def em():
                    xt = xth[0]
                    psv = psA.tile([128, NQ], F32, tag="ps")
                    for tt in range(4):
                        kk = g * 4 + tt
                        for kc in range(KC):
                            nc.tensor.matmul(
                                psv[:, tt * 128 : (tt + 1) * 128],
                                xt[:, kc, kk * 128 : (kk + 1) * 128],
                                wq_sb[:, kc, 2 * CW : 3 * CW],
                                start=(kc == 0), stop=(kc == KC - 1),
                            )
                    # scatter 4 token-tiles into vb's 65-stride head blocks
                    dstv = bass.AP(
                        vb.tensor,
                        vb[:].offset + g * 4 * VST,
                        [vb[:].ap[0], [VST, 4], [HD + 1, HPC], [1, HD]],
                    )
                    srcv = psv[:].rearrange("p (t h d) -> p t h d", t=4, h=HPC)
                    nc.vector.tensor_copy(dstv, srcv)
                return em

            def epilogue():
                # ones columns (denominator trick): col 65*j + HD per block
                onesv = bass.AP(
                    vb.tensor,
                    vb[:].offset + HD,
                    [vb[:].ap[0], [HD + 1, NKK * HPC]],
                )
                nc.vector.tensor_copy(onesv, ones32[:])
                # tail pad so the 128-wide AV ldweights never reads junk
                padv = bass.AP(
                    vb.tensor, vb[:].offset + NKK * VST, [vb[:].ap[0], [1, 64]]
                )
                nc.gpsimd.memset(padv, 0.0)

            ems = [prologue]
            for m in range(2):
                for g in range(2):
                    ems.append(qk_group(m, g))
            for g in range(NKK // 4):
                ems.append(v_group(g))
            ems.append(epilogue)
            return ems

        def attention(b, qt, kt2, vb, yt, fillers=()):
            # flattened (jq, kk) stream, jq descending, software-pipelined
            # one S^T step ahead so the PE queue never head-of-line blocks
            steps = []
            for jq in range(T // NQ - 1, -1, -1):
                for kk in range(NG * (jq + 1)):
                    steps.append((jq, kk))

            def s_step(jq, kk):
                q0 = jq * NQ
                i = kk - NG * jq        # >= 0 on the diagonal run
                c0 = max(i, 0) * 128    # first valid q col in this chunk
                w = NQ - c0
                st = pst.tile([128, HPC * NQ], F32, tag="st")
                for h in range(HPC):
                    nc.tensor.matmul(
                        st[:, h * NQ + c0 : (h + 1) * NQ],
                        kt2[:, kk * 256 + h * 128 : kk * 256 + (h + 1) * 128],
                        qt[:, q0 + c0 : q0 + NQ],
                        start=True,
                        stop=True,
                    )
                return st, c0, w, i

            def e_step(st, c0, w, i):
                ptk = ptpool.tile([128, HPC * NQ], BF16, tag="pt")
                stv = bass.AP(st.tensor, st[:].offset + c0,
                              [st[:].ap[0], [NQ, HPC], [1, w]])
                ptv = bass.AP(ptk.tensor, ptk[:].offset + c0,
                              [ptk[:].ap[0], [NQ, HPC], [1, w]])
                nc.scalar.activation(ptv, stv, EXP, scale=scale)
                if i >= 0:
                    # zero q < kpart inside the 128-wide diagonal block
                    tri = bass.AP(ptk.tensor, ptk[:].offset + c0,
                                  [ptk[:].ap[0], [NQ, HPC], [1, 128]])
                    nc.gpsimd.affine_select(
                        out=tri,
                        in_=tri,
                        pattern=[[0, HPC], [1, 128]],
                        channel_multiplier=-1,
                        base=0,
                        compare_op=mybir.AluOpType.is_ge,
                        fill=0.0,
                    )
                return ptk

            def av_step(avs, ptk, c0, kk, nkk):
                for h in range(HPC):
                    # 128-wide stationary: head h's 65 cols + 63 junk
                    nc.tensor.matmul(
                        avs[h][:, c0:NQ],
                        vb[:, kk * VST + h * (HD + 1) :
                             kk * VST + h * (HD + 1) + 128],
                        ptk[:, h * NQ + c0 : (h + 1) * NQ],
                        start=(kk == 0),
                        stop=(kk == nkk - 1),
                    )

            def norm_evac(avs, jq):
                # rows 0..63 unnormalized O^T, row 64 denominator.
                # y = O * (1/den) fused into the evacuation multiply.
                q0 = jq * NQ
                for h in range(HPC):
                    # custom DVE ops don't honor a nonzero base partition on
                    # the input AP: stage the den row to partition 0 first
                    dnr = dnpool.tile([1, NQ], F32, tag=f"d{h}")
                    nc.vector.tensor_copy(dnr[:], avs[h][HD : HD + 1, :])
                    r = dnpool.tile([1, NQ], F32, tag=f"r{h}")
                    nc.vector.reciprocal_approx_fast(r[:], dnr[:])
                    R = dnpool.tile([HD, NQ], F32, tag=f"R{h}")
                    nc.gpsimd.partition_broadcast(R[:], r[:])
                    nc.vector.tensor_mul(
                        yt[h * HD : (h + 1) * HD, q0 : q0 + NQ],
                        avs[h][0:HD, :],
                        R[:],
                    )

            # spread filler emitters (next batch's qkv bursts, previous
            # batch's out-proj chunks) evenly across the attention steps so
            # ready PE work sits in the queue wherever attention waits on EXP
            fillers = list(fillers)
            nf = 0

            avs = None
            pend = s_step(*steps[0])
            for n, (jq, kk) in enumerate(steps):
                if kk == 0:
                    av0 = pav.tile([128, NQ], F32, tag="av0")
                    av1 = pav.tile([128, NQ], F32, tag="av1")
                    avs = (av0, av1)
                ptk = e_step(*pend)
                c0 = pend[1]
                if n + 1 < len(steps):
                    pend = s_step(*steps[n + 1])
                nkk = NG * (jq + 1)
                av_step(avs, ptk, c0, kk, nkk)
                if kk == nkk - 1:
                    norm_evac(avs, jq)
                want = (n + 1) * len(fillers) // len(steps)
                while nf < want:
                    fillers[nf]()
                    nf += 1
            while nf < len(fillers):
                fillers[nf]()
                nf += 1

        def out_proj_emitters(b, yt):
            """out_proj(b) as 4 chunk emitters of 4 token-tiles each."""
            def chunk(c):
                def em():
                    tok0 = b * T + c * NQ
                    for ts in range(NQ // 128):
                        col = c * NQ + ts * 128
                        os_ = ospool.tile([128, D], BF16, tag="os")
                        for nn in range(D // NQ):
                            pp = psA.tile([128, NQ], F32, tag="ps")
                            nc.tensor.matmul(
                                pp[:],
                                yt[:, col : col + 128],
                                wp_sb[:, nn * NQ : (nn + 1) * NQ],
                                start=True,
                                stop=True,
                            )
                            # split evacuation across DVE and ACT
                            if nn == 0:
                                nc.vector.tensor_copy(
                                    os_[:, nn * NQ : (nn + 1) * NQ], pp[:]
                                )
                            else:
                                nc.scalar.copy(
                                    os_[:, nn * NQ : (nn + 1) * NQ], pp[:]
                                )
                        nc.sync.dma_start(
                            out[tok0 + ts * 128 : tok0 + (ts + 1) * 128, :],
                            os_[:],
                        )
                return em
            return [chunk(c) for c in range(T // NQ)]

        def alloc_tiles():
            qt = qkpool.tile([128, T], BF16, tag="qt")
            kt2 = qkpool.tile([128, 2 * T], BF16, tag="kt2")
            vb = vbpool.tile([128, VBW], BF16, tag="vb")
            yt = ytpool.tile([128, T], BF16, tag="yt")
            return qt, kt2, vb, yt

        # batch 0's qkv runs standalone; afterwards each attention(b) carries
        # qkv(b+1) and out_proj(b-1) interleaved as PE filler
        tiles = {0: alloc_tiles()}
        for em in qkv_emitters(0, *tiles[0][:3]):
            em()
        for b in range(B):
            fa, fb = [], []
            if b + 1 < B:
                tiles[b + 1] = alloc_tiles()
                fa = qkv_emitters(b + 1, *tiles[b + 1][:3])
            if b - 1 >= 0:
                fb = out_proj_emitters(b - 1, tiles[b - 1][3])
            # prologue (DMA) first, then out-proj chunks early (ready
            # immediately, cover the xt DMA latency), then qkv bursts
            fillers = fa[:1] + fb[:2] + fa[1:3] + fb[2:] + fa[3:]
            qt, kt2, vb, yt = tiles[b]
            attention(b, qt, kt2, vb, yt, fillers)
            if b - 2 >= 0:
                del tiles[b - 2]
        for em in out_proj_emitters(B - 1, tiles[B - 1][3]):
            em()


_NC_CACHE = None


def make_in_maps(x, w_attn, w_proj):
    x = np.asarray(x, dtype=np.float32)
    w_attn = np.asarray(w_attn, dtype=np.float32)
    w_proj = np.asarray(w_proj, dtype=np.float32)

    xT = np.ascontiguousarray(x.reshape(BT, D).T).astype(BF)  # [D, BT]

    in_maps = []
    for c in range(NCORES):
        c0 = c * CW
        wq = w_attn[:, c0 : c0 + CW]
        wk = w_attn[:, D + c0 : D + c0 + CW]
        wv = w_attn[:, 2 * D + c0 : 2 * D + c0 + CW]
        wslice = np.concatenate([wq, wk, wv], axis=1)          # [D, 3*CW]
        wpacked = np.ascontiguousarray(
            wslice.reshape(KC, 128, 3 * CW).transpose(1, 0, 2)
        ).reshape(128, KC * 3 * CW).astype(BF)
        wpc = np.ascontiguousarray(w_proj[c0 : c0 + CW, :]).astype(BF)
        in_maps.append({"xT": xT, "wqkv": wpacked, "wp": wpc})
    return in_maps


def kernel(x: np.ndarray, w_attn: np.ndarray, w_proj: np.ndarray) -> np.ndarray:
    global _NC_CACHE
    if _NC_CACHE is None:
        _NC_CACHE = build_kernel()
    nc = _NC_CACHE

    in_maps = make_in_maps(x, w_attn, w_proj)
    res = run_bass_kernel_spmd(nc, in_maps, core_ids=list(range(NCORES)))
    acc = np.zeros((BT, D), dtype=np.float32)
    for r in res.results:
        acc += np.asarray(r["out"], dtype=np.float32)
    return acc.reshape(B, T, D)


if __name__ == "__main__":
    inputs = {
        "x": np.random.randn(B, T, D).astype(np.float32),
        "w_attn": (np.random.randn(D, 3 * D) / np.sqrt(D)).astype(np.float32),
        "w_proj": (np.random.randn(D, D) / np.sqrt(D)).astype(np.float32),
    }
    y = kernel(**inputs)
    print(y.shape, y.dtype)


# revision 19
# speedup vs baseline: 1.1843x; 1.0020x over previous
"""Causal self-attention on 8 NeuronCores (TRN2), tensor-parallel over heads.

Reference: y = proj(softmax(causal(Q K^T / sqrt(64))) V) with
B=4, T=2048, D=1024, H=16 heads, head_dim=64.

Sharding: each core owns 2 heads (a 128-column slice of the Q/K/V
projections and the matching 128 rows of w_proj) for all batches. Each
core emits a partial [B*T, D] output (bf16); the host sums the 8
partials in fp32 (row-parallel matmul unshard) and reshapes to [B,T,D].

Design notes:
  - all matmul operands bf16 (FWL weight loads, half DMA traffic)
  - weight-stationary Q/K projection (2 PSUM banks, LDW amortized)
  - V projected directly token-major (x tiles stationary, w_v moving)
    so no PE transposes are needed for the AV lhsT
  - K^T stored block-diagonal (kt2): head h's 64 d-rows at partition
    rows h*64 of column block 256j+128h, zeros elsewhere, so S^T runs
    as full-K=128 matmuls that qualify for fast weight load
  - softmax normalization deferred: unnormalized O and the denominator
    row come out of the AV matmuls (ones column trick); 1/den via
    reciprocal_approx_fast, partition-broadcast, divide fused into the
    PSUM->SBUF evacuation multiply
  - jq iterated descending so attention starts with the deepest kk run
    (pipeline fills; the shallow jq=0 block lands where out-proj/qkv
    of neighbor batches provide PE filler work)
  - out-proj evacuation split across DVE and ACT
"""

import sys

for _p in ("/opt/trn_rl_repo",):
    if _p not in sys.path:
        sys.path.insert(0, _p)

import ml_dtypes
import numpy as np

import concourse.bass as bass
import concourse.bacc as bacc
import concourse.mybir as mybir
from concourse import tile
from concourse.bass_utils import run_bass_kernel_spmd

B, T, D, H = 4, 2048, 1024, 16
HD = D // H           # 64 head dim
NCORES = 8
HPC = H // NCORES     # 2 heads per core
CW = HPC * HD         # 128: per-core qkv column slice width
BT = B * T            # 8192 tokens
KC = D // 128         # 8 contraction chunks for the qkv projection
NQ = 512              # query chunk
NG = NQ // 128        # 4 key-tiles per S^T group
F32 = mybir.dt.float32
BF16 = mybir.dt.bfloat16
EXP = mybir.ActivationFunctionType.Exp
BF = ml_dtypes.bfloat16

VST = HPC * (HD + 1)  # 130: V tile stride (per head: 64 cols + ones col)
NKK = T // 128        # 16 key tiles per batch
VBW = NKK * VST + 64  # vb width incl. tail pad for the 128-wide AV ldweights


def build_kernel():
    nc = bacc.Bacc("TRN2", target_bir_lowering=False, debug=False)

    xT = nc.dram_tensor("xT", [D, BT], BF16, kind="ExternalInput")
    # wqkv packed on host as [128, KC, 3*CW]: (kc,:) = rows kc*128..+128 of
    # [w_q_slice | w_k_slice | w_v_slice]
    wqkv = nc.dram_tensor("wqkv", [128, KC * 3 * CW], BF16, kind="ExternalInput")
    wp = nc.dram_tensor("wp", [CW, D], BF16, kind="ExternalInput")
    out = nc.dram_tensor("out", [BT, D], BF16, kind="ExternalOutput")

    with tile.TileContext(nc) as tc:
        _body(tc, xT.ap(), wqkv.ap(), wp.ap(), out.ap())
    nc.compile()
    return nc


def _body(tc, xT, wqkv, wp, out):
    nc = tc.nc
    with (
        tc.tile_pool(name="const", bufs=1) as const,
        tc.tile_pool(name="xin", bufs=2) as xin,
        tc.tile_pool(name="qk", bufs=2) as qkpool,
        tc.tile_pool(name="vb", bufs=2) as vbpool,
        tc.tile_pool(name="pt", bufs=3) as ptpool,
        tc.tile_pool(name="yt", bufs=2) as ytpool,
        tc.tile_pool(name="dn", bufs=2) as dnpool,
        tc.tile_pool(name="os", bufs=3) as ospool,
        tc.tile_pool(name="psA", bufs=2, space="PSUM") as psA,
        tc.tile_pool(name="pst", bufs=2, space="PSUM") as pst,
        tc.tile_pool(name="pav", bufs=1, space="PSUM") as pav,
    ):
        # ---- constants ----
        # per-kc weight loads so the first projection matmul can start as
        # soon as (wqkv chunk 0, x chunk 0) land instead of the full 1MB
        wq_sb = const.tile([128, KC, 3 * CW], BF16, tag="wqkv")
        wq_dr = wqkv.rearrange("p (k c) -> p k c", k=KC)
        for kc in range(KC):
            nc.sync.dma_start(wq_sb[:, kc, :], wq_dr[:, kc, :])
        wp_sb = const.tile([CW, D], BF16, tag="wp")
        nc.sync.dma_start(wp_sb[:], wp[:])
        ones32 = const.tile([128, NKK * HPC], BF16, tag="ones32")
        nc.gpsimd.memset(ones32[:], 1.0)
        scale = 1.0 / float(np.sqrt(HD))

        def qkv_emitters(b, qt, kt2, vb):
            """qkv_proj(b) as a list of closures, each emitting one dense
            PE burst (or the DMA prologue), so they can be interleaved into
            the previous batch's attention stream as PE filler work."""
            tok0 = b * T
            xth = []

            def prologue():
                xt = xin.tile([128, KC, T], BF16, tag="xt")
                xth.append(xt)
                # 128KB segments, seg-major: lands one per DMA queue in
                # parallel so the first projection matmul starts early
                for s in range(4):
                    for kc in range(KC):
                        nc.sync.dma_start(
                            xt[:, kc, s * NQ : (s + 1) * NQ],
                            xT[kc * 128 : (kc + 1) * 128,
                               tok0 + s * NQ : tok0 + (s + 1) * NQ],
                        )
                # zero kt2's off-diagonal quadrants (cheap on GpSimd)
                for h in range(HPC):
                    ksl = kt2[h * HD : (h + 1) * HD, :]
                    z = bass.AP(
                        ksl.tensor,
                        ksl.offset + (1 - h) * 128,
                        [ksl.ap[0], [256, NKK], [1, 128]],
                    )
                    nc.gpsimd.memset(z, 0.0)

            def qk_group(m, g):
                def em():
                    xt = xth[0]
                    ps0 = psA.tile([128, NQ], F32, tag="ps")
                    ps1 = psA.tile([128, NQ], F32, tag="ps")
                    t0 = g * 2 * NQ
                    for kc in range(KC):
                        w_ap = wq_sb[:, kc, m * CW : (m + 1) * CW]
                        nc.tensor.matmul(
                            ps0[:], w_ap, xt[:, kc, t0 : t0 + NQ],
                            start=(kc == 0), stop=(kc == KC - 1),
                        )
                        nc.tensor.matmul(
                            ps1[:], w_ap, xt[:, kc, t0 + NQ : t0 + 2 * NQ],
                            start=(kc == 0), stop=(kc == KC - 1),
                        )
                    if m == 0:
                        nc.vector.tensor_copy(qt[:, t0 : t0 + NQ], ps0[:])
                        nc.vector.tensor_copy(qt[:, t0 + NQ : t0 + 2 * NQ], ps1[:])
                    else:
                        # K^T into block-diagonal kt2
                        for ci, ps in ((0, ps0), (1, ps1)):
                            j0 = (t0 + ci * NQ) // 128
                            for h in range(HPC):
                                ksl = kt2[h * HD : (h + 1) * HD, :]
                                dstk = bass.AP(
                                    ksl.tensor,
                                    ksl.offset + j0 * 256 + h * 128,
                                    [ksl.ap[0], [256, NG], [1, 128]],
                                )
                                psl = ps[h * HD : (h + 1) * HD, :]
                                srck = bass.AP(
                                    psl.tensor,
                                    psl.offset,
                                    [psl.ap[0], [128, NG], [1, 128]],
                                )
                                nc.vector.tensor_copy(dstk, srck)
                return em

            def v_group(g):
                def em():
                    xt = xth[0]
                    psv = psA.tile([128, NQ], F32, tag="ps")
                    for tt in range(4):
                        kk = g * 4 + tt
                        for kc in range(KC):
                            nc.tensor.matmul(
                                psv[:, tt * 128 : (tt + 1) * 128],
                                xt[:, kc, kk * 128 : (kk + 1) * 128],
                                wq_sb[:, kc, 2 * CW : 3 * CW],
                                start=(kc == 0), stop=(kc == KC - 1),
                            )
                    # scatter 4 token-tiles into vb's 65-stride head blocks
                    dstv = bass.AP(
                        vb.tensor,
                        vb[:].offset + g * 4 * VST,
                        [vb[:].ap[0], [VST, 4], [HD + 1, HPC], [1, HD]],
                    )
                    srcv = psv[:].rearrange("p (t h d) -> p t h d", t=4, h=HPC)
                    nc.vector.tensor_copy(dstv, srcv)
                return em

            def epilogue():
                # ones columns (denominator trick): col 65*j + HD per block
                onesv = bass.AP(
                    vb.tensor,
                    vb[:].offset + HD,
                    [vb[:].ap[0], [HD + 1, NKK * HPC]],
                )
                nc.vector.tensor_copy(onesv, ones32[:])
                # tail pad so the 128-wide AV ldweights never reads junk
                padv = bass.AP(
                    vb.tensor, vb[:].offset + NKK * VST, [vb[:].ap[0], [1, 64]]
                )
                nc.gpsimd.memset(padv, 0.0)

            ems = [prologue]
            for m in range(2):
                for g in range(2):
                    ems.append(qk_group(m, g))
            for g in range(NKK // 4):
                ems.append(v_group(g))
            ems.append(epilogue)
            return ems

        def attention(b, qt, kt2, vb, yt, fillers=()):
            # flattened (jq, kk) stream, jq descending, software-pipelined
            # one S^T step ahead so the PE queue never head-of-line blocks
            steps = []
            for jq in range(T // NQ - 1, -1, -1):
                for kk in range(NG * (jq + 1)):
                    steps.append((jq, kk))

            def s_step(jq, kk):
                q0 = jq * NQ
                i = kk - NG * jq        # >= 0 on the diagonal run
                c0 = max(i, 0) * 128    # first valid q col in this chunk
                w = NQ - c0
                st = pst.tile([128, HPC * NQ], F32, tag="st")
                for h in range(HPC):
                    nc.tensor.matmul(
                        st[:, h * NQ + c0 : (h + 1) * NQ],
                        kt2[:, kk * 256 + h * 128 : kk * 256 + (h + 1) * 128],
                        qt[:, q0 + c0 : q0 + NQ],
                        start=True,
                        stop=True,
                    )
                return st, c0, w, i

            def e_step(st, c0, w, i):
                ptk = ptpool.tile([128, HPC * NQ], BF16, tag="pt")
                stv = bass.AP(st.tensor, st[:].offset + c0,
                              [st[:].ap[0], [NQ, HPC], [1, w]])
                ptv = bass.AP(ptk.tensor, ptk[:].offset + c0,
                              [ptk[:].ap[0], [NQ, HPC], [1, w]])
                nc.scalar.activation(ptv, stv, EXP, scale=scale)
                if i >= 0:
                    # zero q < kpart inside the 128-wide diagonal block
                    tri = bass.AP(ptk.tensor, ptk[:].offset + c0,
                                  [ptk[:].ap[0], [NQ, HPC], [1, 128]])
                    nc.gpsimd.affine_select(
                        out=tri,
                        in_=tri,
                        pattern=[[0, HPC], [1, 128]],
                        channel_multiplier=-1,
                        base=0,
                        compare_op=mybir.AluOpType.is_ge,
                        fill=0.0,
                    )
                return ptk

            def av_step(avs, ptk, c0, kk, nkk):
                for h in range(HPC):
                    # 128-wide stationary: head h's 65 cols + 63 junk
                    nc.tensor.matmul(
                        avs[h][:, c0:NQ],
                        vb[:, kk * VST + h * (HD + 1) :
                             kk * VST + h * (HD + 1) + 128],
                        ptk[:, h * NQ + c0 : (h + 1) * NQ],
                        start=(kk == 0),
                        stop=(kk == nkk - 1),
                    )

            def norm_evac(avs, jq):
                # rows 0..63 unnormalized O^T, row 64 denominator.
                # Evacuate PSUM fast (releases the av banks for the next jq's
                # accumulation), then normalize from SBUF off the hot path.
                q0 = jq * NQ
                post = []
                for h in range(HPC):
                    ytu = dnpool.tile([HD, NQ], F32, tag=f"y{h}")
                    nc.vector.tensor_copy(ytu[:], avs[h][0:HD, :])
                    # den row staged straight to partition 0 (custom DVE ops
                    # don't honor a nonzero base partition on the input AP)
                    dnr = dnpool.tile([1, NQ], F32, tag=f"d{h}")
                    nc.vector.tensor_copy(dnr[:], avs[h][HD : HD + 1, :])
                    post.append((h, ytu, dnr))
                for h, ytu, dnr in post:
                    r = dnpool.tile([1, NQ], F32, tag=f"r{h}")
                    nc.vector.reciprocal_approx_fast(r[:], dnr[:])
                    R = dnpool.tile([HD, NQ], F32, tag=f"R{h}")
                    nc.gpsimd.partition_broadcast(R[:], r[:])
                    nc.vector.tensor_mul(
                        yt[h * HD : (h + 1) * HD, q0 : q0 + NQ],
                        ytu[:],
                        R[:],
                    )

            # spread filler emitters (next batch's qkv bursts, previous
            # batch's out-proj chunks) evenly across the attention steps so
            # ready PE work sits in the queue wherever attention waits on EXP
            fillers = list(fillers)
            nf = 0

            avs = None
            pend = s_step(*steps[0])
            for n, (jq, kk) in enumerate(steps):
                if kk == 0:
                    av0 = pav.tile([128, NQ], F32, tag="av0")
                    av1 = pav.tile([128, NQ], F32, tag="av1")
                    avs = (av0, av1)
                ptk = e_step(*pend)
                c0 = pend[1]
                if n + 1 < len(steps):
                    pend = s_step(*steps[n + 1])
                nkk = NG * (jq + 1)
                av_step(avs, ptk, c0, kk, nkk)
                if kk == nkk - 1:
                    norm_evac(avs, jq)
                    # a filler right at the jq boundary covers the av-bank
                    # release latency
                    if nf < len(fillers):
                        fillers[nf]()
                        nf += 1
                want = (n + 1) * len(fillers) // len(steps)
                while nf < want:
                    fillers[nf]()
                    nf += 1
            while nf < len(fillers):
                fillers[nf]()
                nf += 1

        def out_proj_emitters(b, yt):
            """out_proj(b) as 4 chunk emitters of 4 token-tiles each."""
            def chunk(c):
                def em():
                    tok0 = b * T + c * NQ
                    for ts in range(NQ // 128):
                        col = c * NQ + ts * 128
                        os_ = ospool.tile([128, D], BF16, tag="os")
                        for nn in range(D // NQ):
                            pp = psA.tile([128, NQ], F32, tag="ps")
                            nc.tensor.matmul(
                                pp[:],
                                yt[:, col : col + 128],
                                wp_sb[:, nn * NQ : (nn + 1) * NQ],
                                start=True,
                                stop=True,
                            )
                            # split evacuation across DVE and ACT
                            if nn == 0:
                                nc.vector.tensor_copy(
                                    os_[:, nn * NQ : (nn + 1) * NQ], pp[:]
                                )
                            else:
                                nc.scalar.copy(
                                    os_[:, nn * NQ : (nn + 1) * NQ], pp[:]
                                )
                        nc.sync.dma_start(
                            out[tok0 + ts * 128 : tok0 + (ts + 1) * 128, :],
                            os_[:],
                        )
                return em
            return [chunk(c) for c in range(T // NQ)]

        def alloc_tiles():
            qt = qkpool.tile([128, T], BF16, tag="qt")
            kt2 = qkpool.tile([128, 2 * T], BF16, tag="kt2")
            vb = vbpool.tile([128, VBW], BF16, tag="vb")
            yt = ytpool.tile([128, T], BF16, tag="yt")
            return qt, kt2, vb, yt

        # batch 0's qkv runs standalone; afterwards each attention(b) carries
        # qkv(b+1) and out_proj(b-1) interleaved as PE filler
        tiles = {0: alloc_tiles()}
        for em in qkv_emitters(0, *tiles[0][:3]):
            em()
        for b in range(B):
            fa, fb = [], []
            if b + 1 < B:
                tiles[b + 1] = alloc_tiles()
                fa = qkv_emitters(b + 1, *tiles[b + 1][:3])
            if b - 1 >= 0:
                fb = out_proj_emitters(b - 1, tiles[b - 1][3])
            # prologue (DMA) first, then out-proj chunks early (ready
            # immediately, cover the xt DMA latency), then qkv bursts
            fillers = fa[:1] + fb[:2] + fa[1:3] + fb[2:] + fa[3:]
            qt, kt2, vb, yt = tiles[b]
            attention(b, qt, kt2, vb, yt, fillers)
            if b - 2 >= 0:
                del tiles[b - 2]
        for em in out_proj_emitters(B - 1, tiles[B - 1][3]):
            em()


_NC_CACHE = None


def make_in_maps(x, w_attn, w_proj):
    x = np.asarray(x, dtype=np.float32)
    w_attn = np.asarray(w_attn, dtype=np.float32)
    w_proj = np.asarray(w_proj, dtype=np.float32)

    xT = np.ascontiguousarray(x.reshape(BT, D).T).astype(BF)  # [D, BT]

    in_maps = []
    for c in range(NCORES):
        c0 = c * CW
        wq = w_attn[:, c0 : c0 + CW]
        wk = w_attn[:, D + c0 : D + c0 + CW]
        wv = w_attn[:, 2 * D + c0 : 2 * D + c0 + CW]
        wslice = np.concatenate([wq, wk, wv], axis=1)          # [D, 3*CW]
        wpacked = np.ascontiguousarray(
            wslice.reshape(KC, 128, 3 * CW).transpose(1, 0, 2)
        ).reshape(128, KC * 3 * CW).astype(BF)
        wpc = np.ascontiguousarray(w_proj[c0 : c0 + CW, :]).astype(BF)
        in_maps.append({"xT": xT, "wqkv": wpacked, "wp": wpc})
    return in_maps


def kernel(x: np.ndarray, w_attn: np.ndarray, w_proj: np.ndarray) -> np.ndarray:
    global _NC_CACHE
    if _NC_CACHE is None:
        _NC_CACHE = build_kernel()
    nc = _NC_CACHE

    in_maps = make_in_maps(x, w_attn, w_proj)
    res = run_bass_kernel_spmd(nc, in_maps, core_ids=list(range(NCORES)))
    acc = np.zeros((BT, D), dtype=np.float32)
    for r in res.results:
        acc += np.asarray(r["out"], dtype=np.float32)
    return acc.reshape(B, T, D)


if __name__ == "__main__":
    inputs = {
        "x": np.random.randn(B, T, D).astype(np.float32),
        "w_attn": (np.random.randn(D, 3 * D) / np.sqrt(D)).astype(np.float32),
        "w_proj": (np.random.randn(D, D) / np.sqrt(D)).astype(np.float32),
    }
    y = kernel(**inputs)
    print(y.shape, y.dtype)
